# revision 1
# baseline (speedup 1.0000x reference)
"""ConnectedComponentContentEncoder — Trainium2 Bass kernel.

Strategy: pure data parallelism over batch B=128 -> 16 samples per core
on 8 NeuronCores.

Host (cheap, int grid only, 460KB): connected-component labeling of the
[B,30,30] int32 color grid + per-object bbox extraction. Converted into a
per-sample bbox mean-pool matrix Mhat [900,16] and bbox feature vectors,
so the device never needs data-dependent control flow.

Device (memory-heavy part, 118MB grid_emb): per sample
  pool_T[d,k] = grid_emb_flat[p,d]^T @ Mhat[p,k]        (PE, streams ge once)
then batched over all 16 samples (256 object slots per core):
  MLP (W1+gelu, W2), valid masking, orthogonal projection against the
  normalized mean structure vector, Wp projection, LayerNorm.
Activations are kept feature-major [d, sample*obj] so weights act as lhsT
with no transposes; row-vector reductions/broadcasts over the feature
(partition) axis are done with ones-vector matmuls on the PE. Final PE
transpose emits [sample*obj, d] for a contiguous DMA to the output.
"""
import sys

sys.path.insert(0, "/opt/trn_rl_repo")

import numpy as np

H, W = 30, 30
D = 256
K = 16           # MAX_OBJECTS
HW = H * W       # 900
SENT = HW
B = 128
NCORES = 8
S = B // NCORES  # 16 samples per core
SO = S * K       # 256 object slots per core
PCHUNK = 1024    # Mhat rows padded to 8*128


# ----------------------------------------------------------------------------
# Host preprocessing: connected components + object bboxes (mirrors reference)
# ----------------------------------------------------------------------------

def _label_components(grid):
    lin = np.arange(HW, dtype=np.int32).reshape(1, H, W)
    fg = grid > 0
    lab = np.where(fg, lin, SENT).astype(np.int32)
    gp = np.pad(grid, ((0, 0), (1, 1), (1, 1)), constant_values=-1)
    nb = grid.shape[0]
    while True:
        lp = np.pad(lab, ((0, 0), (1, 1), (1, 1)), constant_values=SENT)
        m = lab.copy()
        for di, dj in ((-1, 0), (1, 0), (0, -1), (0, 1)):
            ls = lp[:, 1 + di:1 + di + H, 1 + dj:1 + dj + W]
            gs = gp[:, 1 + di:1 + di + H, 1 + dj:1 + dj + W]
            m = np.minimum(m, np.where(gs == grid, ls, SENT))
        m = np.where(fg, m, SENT)
        flat = m.reshape(nb, HW)
        jumped = np.take_along_axis(flat, np.clip(flat, 0, HW - 1), axis=1)
        flat = np.where(flat < SENT, np.minimum(flat, jumped), SENT)
        new = flat.reshape(nb, H, W)
        if np.array_equal(new, lab):
            return new
        lab = new


def _build_masks(grid):
    """grid [B,H,W] int32 -> (mhat [B,1024,K] f32, bboxT [B,5,K] f32,
    validf [B,K] f32). Invalid slots contribute zeros (their embeddings are
    zeroed post-MLP exactly like the reference does)."""
    nb = grid.shape[0]
    labels = _label_components(grid).reshape(nb, HW)
    gf = grid.reshape(nb, HW)
    lin = np.arange(HW, dtype=np.int32)
    rows, cols = lin // W, lin % W
    mhat = np.zeros((nb, HW, K), np.float32)
    bboxT = np.zeros((nb, 5, K), np.float32)
    validf = np.zeros((nb, K), np.float32)
    for b in range(nb):
        l = labels[b]
        roots = np.nonzero((l == lin) & (l < SENT))[0][:K]
        for k, r in enumerate(roots):
            memb = l == r
            rs, cs = rows[memb], cols[memb]
            y, x = int(rs.min()), int(cs.min())
            h = int(rs.max()) + 1 - y
            w = int(cs.max()) + 1 - x
            inb = ((rows >= y) & (rows < y + h) & (cols >= x) & (cols < x + w))
            mhat[b, :HW, k] = inb.astype(np.float32) / float(h * w)
            bboxT[b, :, k] = (gf[b, r] / 9.0, x / float(W), y / float(H),
                              w / float(W), h / float(H))
            validf[b, k] = 1.0
    return mhat, bboxT, validf


def _row_cap(mhat):
    """Smallest multiple of 128 covering every nonzero mhat row (>=128)."""
    nz = np.nonzero(np.abs(mhat).sum(axis=(0, 2)) > 0)[0]
    need = int(nz.max()) + 1 if len(nz) else 1
    return max(128, -(-need // 128) * 128)


# ----------------------------------------------------------------------------
# Device program (built per prefix-cap, SPMD across 8 cores)
# ----------------------------------------------------------------------------

_PROG = {}
GRP = 4  # samples per grid_emb DMA


def _build_program(rcap):
    import concourse.bacc as bacc
    import concourse.mybir as mybir
    import concourse.tile as tile

    f32 = mybir.dt.float32
    nch = rcap // 128
    nc = bacc.Bacc("TRN2", target_bir_lowering=False, debug=False,
                   num_devices=NCORES)

    ge = nc.declare_dram_parameter("ge", [S * HW, D], f32, isOutput=False)
    mh = nc.declare_dram_parameter("mh", [S * rcap, K], f32, isOutput=False)
    wall = nc.declare_dram_parameter("wall", [128, 6 * D], f32, isOutput=False)
    w1c = nc.declare_dram_parameter("w1c", [5, D], f32, isOutput=False)
    bpk = nc.declare_dram_parameter("bpk", [128, 11], f32, isOutput=False)
    spk = nc.declare_dram_parameter("spk", [128, 400], f32, isOutput=False)
    epk = nc.declare_dram_parameter("epk", [S, 768], f32, isOutput=False)
    out = nc.declare_dram_parameter("out", [SO, D], f32, isOutput=True)

    AF = mybir.ActivationFunctionType
    MUL = mybir.AluOpType.mult
    ADD = mybir.AluOpType.add

    with tile.TileContext(nc) as tc:
        with (
            tc.tile_pool(name="const", bufs=1) as cpool,
            tc.tile_pool(name="gp", bufs=2) as gpool,
            tc.tile_pool(name="act", bufs=1) as apool,
            tc.tile_pool(name="scr", bufs=2) as spool,
            tc.tile_pool(name="ps", bufs=2, space="PSUM") as pspool,
            tc.tile_pool(name="pp", bufs=1, space="PSUM") as pppool,
        ):
            # ---- packed constants / weights into SBUF (6 DMAs, off SP) ----
            wallt = cpool.tile([128, 6 * D], f32, tag="wall", name="wall")
            nc.scalar.dma_start(wallt[:], wall[:])
            w1ct = cpool.tile([5, D], f32, tag="w1c", name="w1c")
            nc.scalar.dma_start(w1ct[:], w1c[:])
            bpkt = cpool.tile([128, 11], f32, tag="bpk", name="bpk")
            nc.scalar.dma_start(bpkt[:], bpk[:])
            spkt = cpool.tile([128, 400], f32, tag="spk", name="spk")
            nc.scalar.dma_start(spkt[:], spk[:])
            epkt = cpool.tile([S, 768], f32, tag="epk", name="epk")
            nc.scalar.dma_start(epkt[:], epk[:])
            mtall = cpool.tile([128, S * nch * K], f32, tag="mh", name="mh")
            nc.scalar.dma_start(
                mtall[:],
                mh.rearrange("(s c p) k -> p s c k", s=S, c=nch, p=128))

            w1t = [wallt[:, 0:256], wallt[:, 256:512], w1ct[:]]
            w2t = [wallt[:, 512:768], wallt[:, 768:1024]]
            wpt = [wallt[:, 1024:1280], wallt[:, 1280:1536]]

            def bias_ap(j, m):
                return bpkt[:, m * 5 + j:m * 5 + j + 1]

            orthob = bpkt[:, 10:11]
            selt = spkt[:, 0:16]
            srt = spkt[:, 16:272]
            idt = spkt[:, 272:400]
            et = epkt[:, 0:256]
            vldt = epkt[0:1, 256:512]
            bbxt = epkt[0:5, 512:768]

            onescol = cpool.tile([128, 1], f32, tag="onescol", name="onescol")
            nc.vector.memset(onescol[:], 1.0)
            onesrow = cpool.tile([1, 128], f32, tag="onesrow", name="onesrow")
            nc.vector.memset(onesrow[:], 1.0)

            def bcast_row(row_ap, tag):
                """[1,SO] -> [128,SO] via ones-column matmul."""
                pb = pspool.tile([128, SO], f32, tag="big", name=f"bc_{tag}")
                nc.tensor.matmul(pb[:], onesrow[:], row_ap, start=True,
                                 stop=True)
                sb = spool.tile([128, SO], f32, tag=f"bcs_{tag}",
                                name=f"bcs_{tag}")
                nc.vector.tensor_copy(sb[:], pb[:])
                return sb

            validb = bcast_row(vldt, "vld")

            # ---- structure branch: s_mean and 1/max(|s|^2, eps^2) ----------
            psn = pspool.tile([S, D], f32, tag="row", name="ssum")
            nc.tensor.matmul(psn[:], selt, srt, start=True, stop=True)
            st = spool.tile([S, D], f32, tag="smean", name="smean")
            nc.vector.tensor_scalar_mul(st[:], psn[:], 0.125)
            sq = spool.tile([S, D], f32, tag="ssq", name="ssq")
            nc.vector.tensor_mul(sq[:], st[:], st[:])
            rs = spool.tile([S, 1], f32, tag="srs", name="srs")
            nc.vector.reduce_sum(rs[:], sq[:], axis=mybir.AxisListType.X)
            nc.vector.tensor_scalar_max(rs[:], rs[:], 1e-16)
            rq = spool.tile([S, 1], f32, tag="srq", name="srq")
            nc.vector.reciprocal(rq[:], rs[:])
            prq = pspool.tile([1, SO], f32, tag="row", name="rqrow")
            nc.tensor.matmul(prq[:], rq[:], et, start=True, stop=True)
            rqr = spool.tile([1, SO], f32, tag="rqr", name="rqr")
            nc.vector.tensor_copy(rqr[:], prq[:])
            snr = []
            for dc in range(2):
                prep = pspool.tile([128, SO], f32, tag="big", name="snrep")
                nc.tensor.matmul(prep[:], st[:, dc * 128:(dc + 1) * 128],
                                 et, start=True, stop=True)
                sb = spool.tile([128, SO], f32, tag=f"snr{dc}",
                                name=f"snr{dc}")
                nc.vector.tensor_copy(sb[:], prep[:])
                snr.append(sb)

            # ---- pooling: psum-slice accumulation, one copy per d-chunk ----
            ctp = [pppool.tile([128, SO], f32, tag=f"ctp{dc}", name=f"ctp{dc}")
                   for dc in range(2)]
            gev = ge.rearrange("(s r) d -> r s d", s=S)
            for grp in range(S // GRP):
                gt = gpool.tile([128, GRP * nch * 256], f32, tag="g", name="g")
                gtv = gt[:].rearrange("p (s c d) -> p s c d", s=GRP, c=nch)
                for ci in range(nch):
                    nc.sync.dma_start(
                        gtv[:, :, ci, :],
                        gev[ci * 128:(ci + 1) * 128,
                            grp * GRP:(grp + 1) * GRP, :])
                for si in range(GRP):
                    s = grp * GRP + si
                    for dc in range(2):
                        for ci in range(nch):
                            nc.tensor.matmul(
                                ctp[dc][:, s * K:(s + 1) * K],
                                gt[:, (si * nch + ci) * 256 + dc * 128:
                                   (si * nch + ci) * 256 + (dc + 1) * 128],
                                mtall[:, (s * nch + ci) * K:
                                      (s * nch + ci + 1) * K],
                                start=(ci == 0), stop=(ci == nch - 1))
            ct = []
            for dc in range(2):
                cb = apool.tile([128, SO], f32, tag=f"ct{dc}", name=f"ct{dc}")
                nc.vector.tensor_copy(cb[:], ctp[dc][:])
                ct.append(cb)

            # ---- MLP: hdn = gelu(W1^T @ combined + b1) ---------------------
            ht = []
            for m in range(2):
                ph = pspool.tile([128, SO], f32, tag="big", name="mlp_h")
                nc.tensor.matmul(ph[:], w1t[0][:, m * 128:(m + 1) * 128],
                                 ct[0][:], start=True, stop=False)
                nc.tensor.matmul(ph[:], w1t[1][:, m * 128:(m + 1) * 128],
                                 ct[1][:], start=False, stop=False)
                nc.tensor.matmul(ph[:], w1t[2][:, m * 128:(m + 1) * 128],
                                 bbxt, start=False, stop=True)
                hb = apool.tile([128, SO], f32, tag=f"h{m}", name=f"h{m}")
                nc.scalar.activation(hb[:], ph[:], AF.Gelu, bias=bias_ap(0, m))
                ht.append(hb)

            # ---- obj = W2^T @ hdn + b2, zero invalid slots -----------------
            objt = []
            for m in range(2):
                po = pspool.tile([128, SO], f32, tag="big", name="mlp_o")
                nc.tensor.matmul(po[:], w2t[0][:, m * 128:(m + 1) * 128],
                                 ht[0][:], start=True, stop=False)
                nc.tensor.matmul(po[:], w2t[1][:, m * 128:(m + 1) * 128],
                                 ht[1][:], start=False, stop=True)
                ob = apool.tile([128, SO], f32, tag=f"obj{m}", name=f"obj{m}")
                nc.vector.tensor_scalar_add(ob[:], po[:], bias_ap(1, m))
                nc.vector.tensor_mul(ob[:], ob[:], validb[:])
                objt.append(ob)

            # ---- dot' = sum_d obj*s ; co = (obj - dot'*rq*s) * ortho -------
            pd = pspool.tile([1, SO], f32, tag="row", name="dot")
            for dc in range(2):
                tm = spool.tile([128, SO], f32, tag="dotmul", name="dotmul",
                                bufs=2)
                nc.vector.tensor_mul(tm[:], objt[dc][:], snr[dc][:])
                nc.tensor.matmul(pd[:], onescol[:], tm[:],
                                 start=(dc == 0), stop=(dc == 1))
            dotq = spool.tile([1, SO], f32, tag="dotq", name="dotq")
            nc.vector.tensor_mul(dotq[:], pd[:], rqr[:])
            dotb = bcast_row(dotq[:], "dot")
            cot = []
            for dc in range(2):
                cb = apool.tile([128, SO], f32, tag=f"co{dc}", name=f"co{dc}")
                nc.vector.tensor_mul(cb[:], dotb[:], snr[dc][:])
                nc.vector.tensor_sub(cb[:], objt[dc][:], cb[:])
                nc.vector.tensor_scalar_mul(cb[:], cb[:], orthob)
                cot.append(cb)

            # ---- co = Wp^T @ co + bp ---------------------------------------
            cpt = []
            for m in range(2):
                pc = pspool.tile([128, SO], f32, tag="big", name="mlp_p")
                nc.tensor.matmul(pc[:], wpt[0][:, m * 128:(m + 1) * 128],
                                 cot[0][:], start=True, stop=False)
                nc.tensor.matmul(pc[:], wpt[1][:, m * 128:(m + 1) * 128],
                                 cot[1][:], start=False, stop=True)
                cb = apool.tile([128, SO], f32, tag=f"cp{m}", name=f"cp{m}")
                nc.vector.tensor_scalar_add(cb[:], pc[:], bias_ap(2, m))
                cpt.append(cb)

            # ---- LayerNorm over feature axis (partition dim) ---------------
            ps1 = pspool.tile([1, SO], f32, tag="row", name="s1")
            ps2 = pspool.tile([1, SO], f32, tag="row", name="s2")
            for m in range(2):
                nc.tensor.matmul(ps1[:], onescol[:], cpt[m][:],
                                 start=(m == 0), stop=(m == 1))
            sqs = []
            for m in range(2):
                qq = spool.tile([128, SO], f32, tag="lnsq", name="lnsq",
                                bufs=2)
                nc.vector.tensor_mul(qq[:], cpt[m][:], cpt[m][:])
                sqs.append(qq)
            for m in range(2):
                nc.tensor.matmul(ps2[:], onescol[:], sqs[m][:],
                                 start=(m == 0), stop=(m == 1))
            mur = spool.tile([1, SO], f32, tag="mur", name="mur")
            nc.vector.tensor_scalar_mul(mur[:], ps1[:], 1.0 / D)
            msr = spool.tile([1, SO], f32, tag="msr", name="msr")
            nc.vector.tensor_scalar_mul(msr[:], ps2[:], 1.0 / D)
            varr = spool.tile([1, SO], f32, tag="varr", name="varr")
            nc.vector.tensor_mul(varr[:], mur[:], mur[:])
            nc.vector.tensor_sub(varr[:], msr[:], varr[:])
            nc.vector.tensor_scalar_add(varr[:], varr[:], 1e-5)
            nc.scalar.activation(varr[:], varr[:], AF.Sqrt)
            rstd = spool.tile([1, SO], f32, tag="rstd", name="rstd")
            nc.vector.reciprocal(rstd[:], varr[:])
            mub = bcast_row(mur[:], "mu")
            rstdb = bcast_row(rstd[:], "rstd")
            yt = []
            for m in range(2):
                yb = apool.tile([128, SO], f32, tag=f"y{m}", name=f"y{m}")
                nc.vector.tensor_sub(yb[:], cpt[m][:], mub[:])
                nc.vector.tensor_mul(yb[:], yb[:], rstdb[:])
                nc.vector.tensor_scalar(yb[:], yb[:], bias_ap(3, m),
                                        bias_ap(4, m), op0=MUL, op1=ADD)
                yt.append(yb)

            # ---- transpose [d, so] -> [so, d] and store --------------------
            for q in range(2):
                yo = spool.tile([128, D], f32, tag="yo", name="yo", bufs=2)
                for m in range(2):
                    pt = pspool.tile([128, 128], f32, tag="big", name="tr")
                    nc.tensor.transpose(
                        pt[:], yt[m][:, q * 128:(q + 1) * 128], idt)
                    nc.vector.tensor_copy(yo[:, m * 128:(m + 1) * 128], pt[:])
                nc.sync.dma_start(out[q * 128:(q + 1) * 128, :], yo[:])

    nc.compile()
    return nc


def _get_program(rcap):
    if rcap not in _PROG:
        _PROG[rcap] = _build_program(rcap)
    return _PROG[rcap]


# ----------------------------------------------------------------------------
# Entry point
# ----------------------------------------------------------------------------

def _make_in_maps(np_inputs, mhat, bboxT, validf, rcap):
    grid_emb = np.asarray(np_inputs["grid_emb"], np.float32)
    structure_rep = np.asarray(np_inputs["structure_rep"], np.float32)
    W1 = np.asarray(np_inputs["W1"], np.float32)
    W2 = np.asarray(np_inputs["W2"], np.float32)
    Wp = np.asarray(np_inputs["Wp"], np.float32)
    b1 = np.asarray(np_inputs["b1"], np.float32)
    b2 = np.asarray(np_inputs["b2"], np.float32)
    bp = np.asarray(np_inputs["bp"], np.float32)
    gamma = np.asarray(np_inputs["gamma"], np.float32)
    beta = np.asarray(np_inputs["beta"], np.float32)
    orth = float(np.asarray(np_inputs["ortho_scale"]).reshape(-1)[0])

    wall = np.concatenate([W1[0:128], W1[128:256], W2[0:128], W2[128:256],
                           Wp[0:128], Wp[128:256]], axis=1)
    w1c = np.ascontiguousarray(W1[256:261])
    bpk = np.zeros((128, 11), np.float32)
    for m in range(2):
        for j, vec in enumerate((b1, b2, bp, gamma, beta)):
            bpk[:, m * 5 + j] = vec[m * 128:(m + 1) * 128]
    bpk[:, 10] = orth

    econ = np.zeros((S, SO), np.float32)
    for s in range(S):
        econ[s, s * K:(s + 1) * K] = 1.0
    selc = np.zeros((S * 8, S), np.float32)
    for s in range(S):
        selc[s * 8:(s + 1) * 8, s] = 1.0
    ident = np.eye(128, dtype=np.float32)

    in_maps = []
    for c in range(NCORES):
        sl = slice(c * S, (c + 1) * S)
        spkc = np.concatenate(
            [selc, structure_rep[sl].reshape(S * 8, D), ident], axis=1)
        epkc = np.zeros((S, 768), np.float32)
        epkc[:, 0:256] = econ
        epkc[0, 256:512] = validf[sl].reshape(SO)
        epkc[0:5, 512:768] = bboxT[sl].transpose(1, 0, 2).reshape(5, SO)
        in_maps.append(dict(
            ge=np.ascontiguousarray(grid_emb[sl].reshape(S * HW, D)),
            mh=np.ascontiguousarray(mhat[sl, :rcap].reshape(S * rcap, K)),
            wall=np.ascontiguousarray(wall),
            w1c=w1c, bpk=bpk,
            spk=np.ascontiguousarray(spkc),
            epk=epkc,
        ))
    return in_maps


def kernel(grid_emb, grid, structure_rep, W1, b1, W2, b2, Wp, bp,
           gamma, beta, ortho_scale):
    from concourse.bass_utils import run_bass_kernel_spmd

    np_inputs = dict(grid_emb=grid_emb, grid=grid,
                     structure_rep=structure_rep, W1=W1, b1=b1, W2=W2, b2=b2,
                     Wp=Wp, bp=bp, gamma=gamma, beta=beta,
                     ortho_scale=ortho_scale)
    grid = np.asarray(grid, np.int32)
    mhat, bboxT, validf = _build_masks(grid)
    rcap = _row_cap(mhat)
    in_maps = _make_in_maps(np_inputs, mhat, bboxT, validf, rcap)

    nc = _get_program(rcap)
    res = run_bass_kernel_spmd(nc, in_maps, list(range(NCORES)))
    outs = [res.results[c]["out"].reshape(S, K, D) for c in range(NCORES)]
    return np.concatenate(outs, axis=0)



# revision 4
# speedup vs baseline: 2.6985x; 2.6985x over previous
"""ConnectedComponentContentEncoder — Trainium2 Bass kernel (v2).

Data parallel over batch B=128 -> 16 samples per core on 8 NeuronCores.

Host (cheap, int grid + small tensors): connected-component labeling,
per-object bboxes, and the key observation that objects are tiny — only
the grid positions covered by some bbox (~20-60 rows of 900 per sample)
ever contribute to the pooling. Those rows are gathered host-side into a
dense packed stream per core (fp16), along with a packed block-diagonal
mask matrix [pos, 256 slots] carrying the 1/(h*w) mean-pool weights.
The structure-projector branch is folded into two per-sample vectors
U = s_mean and V = s_mean/max(||s||,1e-8)^2 (ortho_scale folded into Wp).

Device per core (all matmuls fp16, PSUM fp32):
  pool^T[d, slot] = sum_c ge_chunk[c]^T @ mask_chunk[c]    (C chunks of 128)
  MLP (W1+gelu+b1, W2+b2) in feature-major [d, slot] layout,
  ortho projection via ones-matmul dot + broadcast matmul,
  Wp applied with the activations as the *stationary* operand so the
  output lands slot-major [slot, d] — no PE transposes — then LayerNorm
  with per-partition (per-slot) scalars and a direct [256,256] store.
"""
import sys

sys.path.insert(0, "/opt/trn_rl_repo")

import numpy as np

H, W = 30, 30
D = 256
K = 16           # MAX_OBJECTS
HW = H * W       # 900
SENT = HW
B = 128
NCORES = 8
S = B // NCORES  # 16 samples per core
SO = S * K       # 256 object slots per core


# ----------------------------------------------------------------------------
# Host preprocessing: connected components + object bboxes (mirrors reference)
# ----------------------------------------------------------------------------

def _label_components(grid):
    lin = np.arange(HW, dtype=np.int32).reshape(1, H, W)
    fg = grid > 0
    lab = np.where(fg, lin, SENT).astype(np.int32)
    gp = np.pad(grid, ((0, 0), (1, 1), (1, 1)), constant_values=-1)
    nb = grid.shape[0]
    while True:
        lp = np.pad(lab, ((0, 0), (1, 1), (1, 1)), constant_values=SENT)
        m = lab.copy()
        for di, dj in ((-1, 0), (1, 0), (0, -1), (0, 1)):
            ls = lp[:, 1 + di:1 + di + H, 1 + dj:1 + dj + W]
            gs = gp[:, 1 + di:1 + di + H, 1 + dj:1 + dj + W]
            m = np.minimum(m, np.where(gs == grid, ls, SENT))
        m = np.where(fg, m, SENT)
        flat = m.reshape(nb, HW)
        jumped = np.take_along_axis(flat, np.clip(flat, 0, HW - 1), axis=1)
        flat = np.where(flat < SENT, np.minimum(flat, jumped), SENT)
        new = flat.reshape(nb, H, W)
        if np.array_equal(new, lab):
            return new
        lab = new


def _build_masks(grid):
    """grid [B,H,W] int32 -> (mhat [B,900,K] f32 pool weights, bboxT [B,5,K]
    f32 features, validf [B,K] f32)."""
    nb = grid.shape[0]
    labels = _label_components(grid).reshape(nb, HW)
    gf = grid.reshape(nb, HW)
    lin = np.arange(HW, dtype=np.int32)
    rows, cols = lin // W, lin % W
    mhat = np.zeros((nb, HW, K), np.float32)
    bboxT = np.zeros((nb, 5, K), np.float32)
    validf = np.zeros((nb, K), np.float32)
    for b in range(nb):
        l = labels[b]
        roots = np.nonzero((l == lin) & (l < SENT))[0][:K]
        for k, r in enumerate(roots):
            memb = l == r
            rs, cs = rows[memb], cols[memb]
            y, x = int(rs.min()), int(cs.min())
            h = int(rs.max()) + 1 - y
            w = int(cs.max()) + 1 - x
            inb = ((rows >= y) & (rows < y + h) & (cols >= x) & (cols < x + w))
            mhat[b, :, k] = inb.astype(np.float32) / float(h * w)
            bboxT[b, :, k] = (gf[b, r] / 9.0, x / float(W), y / float(H),
                              w / float(W), h / float(H))
            validf[b, k] = 1.0
    return mhat, bboxT, validf


def _expand_uv(uc):
    """[S,256] per-sample vectors -> [128, 2*SO] feature-major broadcast:
    out[p, dc*SO + slot] = uc[slot//K, dc*128 + p]."""
    t = np.repeat(uc, K, axis=0)                      # [SO, 256]
    return t.T.reshape(2, 128, SO).transpose(1, 0, 2).reshape(128, 2 * SO)


def _prepare(np_inputs):
    """Host pack. Returns (key, in_maps)."""
    f16 = np.float16
    grid = np.asarray(np_inputs["grid"], np.int32)
    ge = np.asarray(np_inputs["grid_emb"], np.float32).reshape(B, HW, D)
    sr = np.asarray(np_inputs["structure_rep"], np.float32)
    W1 = np.asarray(np_inputs["W1"], np.float32)
    W2 = np.asarray(np_inputs["W2"], np.float32)
    Wp = np.asarray(np_inputs["Wp"], np.float32)
    b1 = np.asarray(np_inputs["b1"], np.float32)
    b2 = np.asarray(np_inputs["b2"], np.float32)
    bp = np.asarray(np_inputs["bp"], np.float32)
    gamma = np.asarray(np_inputs["gamma"], np.float32)
    beta = np.asarray(np_inputs["beta"], np.float32)
    orth = float(np.asarray(np_inputs["ortho_scale"]).reshape(-1)[0])

    mhat, bboxT, validf = _build_masks(grid)
    rows = [np.nonzero(mhat[b].any(axis=1))[0] for b in range(B)]
    pc = [sum(len(rows[b]) for b in range(c * S, (c + 1) * S))
          for c in range(NCORES)]
    C = max(1, max(-(-p // 128) for p in pc))

    allvalid = bool(validf.min() >= 1.0)
    g1b0 = bool(np.all(gamma == 1.0) and np.all(beta == 0.0))
    key = (C, allvalid, g1b0)

    # structure branch folded to per-sample U, V
    s = sr.mean(axis=1)                               # [B, 256]
    nrm = np.maximum(np.linalg.norm(s, axis=1), 1e-8)
    U = s
    V = s / (nrm ** 2)[:, None]

    # shared weights
    Wpp = Wp * orth
    w1c = np.zeros((128, 256), np.float32)
    w1c[0:5] = W1[256:261]
    wall = np.concatenate(
        [W1[0:128], W1[128:256], W2[0:128], W2[128:256],
         Wpp[0:128], Wpp[128:256], w1c], axis=1).astype(f16)  # [128, 1792]
    bpk = np.zeros((128, 4), np.float32)
    bpk[:, 0] = b1[0:128]
    bpk[:, 1] = b1[128:256]
    bpk[:, 2] = b2[0:128]
    bpk[:, 3] = b2[128:256]

    in_maps = []
    for c in range(NCORES):
        bs = list(range(c * S, (c + 1) * S))
        gep = np.zeros((C * 128, D), np.float32)
        mkp = np.zeros((C * 128, SO), np.float32)
        off = 0
        for si, b in enumerate(bs):
            r = rows[b]
            n = len(r)
            if n:
                gep[off:off + n] = ge[b, r]
                mkp[off:off + n, si * K:(si + 1) * K] = mhat[b, r]
            off += n
        gepk = gep.reshape(C, 128, D).transpose(1, 0, 2).reshape(128, C * D)
        mkpk = mkp.reshape(C, 128, SO).transpose(1, 0, 2).reshape(128, C * SO)
        gm = np.ascontiguousarray(
            np.concatenate([gepk, mkpk], axis=1)).astype(f16)

        bbx = np.zeros((128, SO), np.float32)
        bbx[0:5] = bboxT[bs].transpose(1, 0, 2).reshape(5, SO)
        bprow = np.zeros((128, SO), np.float32)
        bprow[0] = bp
        cpk = np.ascontiguousarray(np.concatenate(
            [_expand_uv(U[bs]), _expand_uv(V[bs]), bbx, bprow],
            axis=1)).astype(f16)                      # [128, 1536]

        im = dict(gm=gm, wall=wall, cpk=cpk, bpk=bpk)
        if not allvalid:
            vrow = validf[bs].reshape(SO)
            im["vrep"] = np.ascontiguousarray(
                np.broadcast_to(np.concatenate([vrow, vrow]),
                                (128, 2 * SO))).astype(f16)
        if not g1b0:
            gb = np.zeros((128, 512), np.float32)
            gb[:, 0:256] = gamma
            gb[:, 256:512] = beta
            im["gb"] = gb
        in_maps.append(im)
    return key, in_maps


# ----------------------------------------------------------------------------
# Device program (built per (C, allvalid, g1b0), SPMD across 8 cores)
# ----------------------------------------------------------------------------

_PROG = {}


def _build_program(key):
    C, allvalid, g1b0 = key
    import concourse.bacc as bacc
    import concourse.mybir as mybir
    import concourse.tile as tile

    f32 = mybir.dt.float32
    f16 = mybir.dt.float16
    AF = mybir.ActivationFunctionType
    MUL = mybir.AluOpType.mult
    SUB = mybir.AluOpType.subtract
    CD = C * D

    nc = bacc.Bacc("TRN2", target_bir_lowering=False, debug=False,
                   num_devices=NCORES)

    gm = nc.declare_dram_parameter("gm", [128, 2 * CD], f16, isOutput=False)
    wall = nc.declare_dram_parameter("wall", [128, 1792], f16, isOutput=False)
    cpk = nc.declare_dram_parameter("cpk", [128, 1536], f16, isOutput=False)
    bpk = nc.declare_dram_parameter("bpk", [128, 4], f32, isOutput=False)
    if not allvalid:
        vrep = nc.declare_dram_parameter("vrep", [128, 2 * SO], f16,
                                         isOutput=False)
    if not g1b0:
        gbp = nc.declare_dram_parameter("gb", [128, 512], f32, isOutput=False)
    out = nc.declare_dram_parameter("out", [SO, D], f32, isOutput=True)

    with tile.TileContext(nc) as tc:
        with (
            tc.tile_pool(name="const", bufs=1) as cpool,
            tc.tile_pool(name="act", bufs=1) as apool,
            tc.tile_pool(name="scr", bufs=1) as spool,
            tc.tile_pool(name="plp", bufs=1, space="PSUM") as plpool,
            tc.tile_pool(name="mmp", bufs=2, space="PSUM") as mmpool,
            tc.tile_pool(name="bcp", bufs=1, space="PSUM") as bcpool,
        ):
            # ---- DMAs ------------------------------------------------------
            gmt = cpool.tile([128, 2 * CD], f16, tag="gm", name="gm")
            nc.sync.dma_start(gmt[:], gm[:])
            wallt = cpool.tile([128, 1792], f16, tag="wall", name="wall")
            nc.scalar.dma_start(wallt[:], wall[:])
            cpkt = cpool.tile([128, 1536], f16, tag="cpk", name="cpk")
            nc.scalar.dma_start(cpkt[:], cpk[:])
            bpkt = cpool.tile([128, 4], f32, tag="bpk", name="bpk")
            nc.scalar.dma_start(bpkt[:], bpk[:])
            if not allvalid:
                vrt = cpool.tile([128, 2 * SO], f16, tag="vr", name="vr")
                nc.vector.dma_start(vrt[:], vrep[:])
            if not g1b0:
                gbt = cpool.tile([128, 512], f32, tag="gb", name="gb")
                nc.vector.dma_start(gbt[:], gbp[:])
            onec = cpool.tile([128, 1], f16, tag="onec", name="onec")
            nc.vector.memset(onec[:], 1.0)
            oner = cpool.tile([1, 128], f16, tag="oner", name="oner")
            nc.vector.memset(oner[:], 1.0)
            epsc = cpool.tile([128, 1], f32, tag="epsc", name="epsc")
            nc.vector.memset(epsc[:], 1e-5)

            # ---- pooling: pool^T[d, slot], accumulate C chunks -------------
            pl = [plpool.tile([128, SO], f32, tag=f"pl{dc}", name=f"pl{dc}")
                  for dc in range(2)]
            for dc in range(2):
                for c in range(C):
                    nc.tensor.matmul(
                        pl[dc][:],
                        gmt[:, c * D + dc * 128: c * D + dc * 128 + 128],
                        gmt[:, CD + c * SO: CD + (c + 1) * SO],
                        start=(c == 0), stop=(c == C - 1))
            comb = apool.tile([128, 2 * SO], f16, tag="comb", name="comb")
            for dc in range(2):
                nc.scalar.copy(comb[:, dc * SO:(dc + 1) * SO], pl[dc][:])

            # ---- MLP1: hdn = gelu(W1^T @ [pool; bbox] + b1) ----------------
            hdn = apool.tile([128, 2 * SO], f16, tag="hdn", name="hdn")
            for m in range(2):
                ph = mmpool.tile([128, SO], f32, tag="mm", name=f"ph{m}")
                nc.tensor.matmul(ph[:], wallt[:, m * 128:(m + 1) * 128],
                                 comb[:, 0:SO], start=True, stop=False)
                nc.tensor.matmul(ph[:], wallt[:, 256 + m * 128:
                                               256 + (m + 1) * 128],
                                 comb[:, SO:2 * SO], start=False, stop=False)
                nc.tensor.matmul(ph[:], wallt[0:5, 1536 + m * 128:
                                              1536 + (m + 1) * 128],
                                 cpkt[0:5, 1024:1280],
                                 start=False, stop=True)
                nc.scalar.activation(hdn[:, m * SO:(m + 1) * SO], ph[:],
                                     AF.Gelu, bias=bpkt[:, m:m + 1])

            # ---- obj = W2^T @ hdn + b2 (masked if any invalid slot) --------
            objsb = apool.tile([128, 2 * SO], f16, tag="obj", name="obj")
            for m in range(2):
                po = mmpool.tile([128, SO], f32, tag="mm", name=f"po{m}")
                nc.tensor.matmul(po[:], wallt[:, 512 + m * 128:
                                              512 + (m + 1) * 128],
                                 hdn[:, 0:SO], start=True, stop=False)
                nc.tensor.matmul(po[:], wallt[:, 768 + m * 128:
                                              768 + (m + 1) * 128],
                                 hdn[:, SO:2 * SO], start=False, stop=True)
                nc.vector.tensor_scalar_add(objsb[:, m * SO:(m + 1) * SO],
                                            po[:], bpkt[:, 2 + m:3 + m])
            if not allvalid:
                nc.vector.tensor_mul(objsb[:], objsb[:], vrt[:])

            # ---- ortho: co = obj - (sum_d obj*U) * V -----------------------
            tm = spool.tile([128, 2 * SO], f16, tag="tm", name="tm")
            nc.vector.tensor_mul(tm[:], objsb[:], cpkt[:, 0:512])
            dotp = bcpool.tile([1, SO], f32, tag="dot", name="dot")
            nc.tensor.matmul(dotp[:], onec[:], tm[:, 0:SO],
                             start=True, stop=False)
            nc.tensor.matmul(dotp[:], onec[:], tm[:, SO:2 * SO],
                             start=False, stop=True)
            dotsb = spool.tile([1, 2 * SO], f16, tag="dotsb", name="dotsb")
            nc.vector.tensor_copy(dotsb[:, 0:SO], dotp[:])
            nc.vector.tensor_copy(dotsb[:, SO:2 * SO], dotp[:])
            dotb = bcpool.tile([128, 2 * SO], f32, tag="dotb", name="dotb")
            nc.tensor.matmul(dotb[:], oner[:], dotsb[:], start=True, stop=True)
            m1 = spool.tile([128, 2 * SO], f16, tag="m1", name="m1")
            nc.vector.tensor_mul(m1[:], dotb[:], cpkt[:, 512:1024])
            cosb = apool.tile([128, 2 * SO], f16, tag="cosb", name="cosb")
            nc.vector.tensor_sub(cosb[:], objsb[:], m1[:])

            # ---- Wp (activations stationary -> slot-major out) + LN --------
            stats = spool.tile([128, 8], f32, tag="stats", name="stats")
            junk = spool.tile([128, 2 * SO], f16, tag="junk", name="junk")
            yt = spool.tile([128, 2 * D], f32, tag="yt", name="yt")
            wq = []
            for q in range(2):
                pw = mmpool.tile([128, D], f32, tag="mm", name=f"pw{q}")
                nc.tensor.matmul(pw[:], cosb[:, q * 128: q * 128 + 128],
                                 wallt[:, 1024:1280], start=True, stop=False)
                nc.tensor.matmul(pw[:], cosb[:, SO + q * 128:
                                              SO + q * 128 + 128],
                                 wallt[:, 1280:1536], start=False, stop=False)
                nc.tensor.matmul(pw[:], oner[:], cpkt[0:1, 1280:1536],
                                 start=False, stop=True)
                wq.append(pw)
                nc.vector.reduce_sum(stats[:, q:q + 1], pw[:],
                                     axis=mybir.AxisListType.X)
                nc.scalar.activation(junk[:, q * D:(q + 1) * D], pw[:],
                                     AF.Square,
                                     accum_out=stats[:, 2 + q:3 + q])
            # mu = sum/D ; var = ssq/D - mu^2 ; rstd = 1/sqrt(var + 1e-5)
            nc.vector.tensor_scalar_mul(stats[:, 4:6], stats[:, 0:2], 1.0 / D)
            nc.vector.tensor_scalar_mul(stats[:, 6:8], stats[:, 2:4], 1.0 / D)
            nc.vector.tensor_mul(stats[:, 0:2], stats[:, 4:6], stats[:, 4:6])
            nc.vector.tensor_sub(stats[:, 2:4], stats[:, 6:8], stats[:, 0:2])
            nc.scalar.activation(stats[:, 6:8], stats[:, 2:4], AF.Sqrt,
                                 bias=epsc[:])
            nc.vector.reciprocal(stats[:, 2:4], stats[:, 6:8])
            for q in range(2):
                nc.vector.tensor_scalar(yt[:, q * D:(q + 1) * D], wq[q][:],
                                        stats[:, 4 + q:5 + q],
                                        stats[:, 2 + q:3 + q],
                                        op0=SUB, op1=MUL)
                if not g1b0:
                    nc.vector.tensor_mul(yt[:, q * D:(q + 1) * D],
                                         yt[:, q * D:(q + 1) * D],
                                         gbt[:, 0:256])
                    nc.vector.tensor_add(yt[:, q * D:(q + 1) * D],
                                         yt[:, q * D:(q + 1) * D],
                                         gbt[:, 256:512])
                nc.sync.dma_start(out[q * 128:(q + 1) * 128, :],
                                  yt[:, q * D:(q + 1) * D])

    nc.compile()
    return nc


def _get_program(key):
    if key not in _PROG:
        _PROG[key] = _build_program(key)
    return _PROG[key]


# ----------------------------------------------------------------------------
# Entry point
# ----------------------------------------------------------------------------

def kernel(grid_emb, grid, structure_rep, W1, b1, W2, b2, Wp, bp,
           gamma, beta, ortho_scale):
    from concourse.bass_utils import run_bass_kernel_spmd

    np_inputs = dict(grid_emb=grid_emb, grid=grid,
                     structure_rep=structure_rep, W1=W1, b1=b1, W2=W2, b2=b2,
                     Wp=Wp, bp=bp, gamma=gamma, beta=beta,
                     ortho_scale=ortho_scale)
    key, in_maps = _prepare(np_inputs)
    nc = _get_program(key)
    res = run_bass_kernel_spmd(nc, in_maps, list(range(NCORES)))
    outs = [res.results[c]["out"].reshape(S, K, D) for c in range(NCORES)]
    return np.concatenate(outs, axis=0)
